# revision 8
# baseline (speedup 1.0000x reference)
"""Trainium2 Bass kernel for the 2-layer CIN (compressed interaction network).

Reference computation (per batch element b, embedding channel d):
  z0[hf=h*40+f]  = x[b,h,d] * x[b,f,d]              (h,f in 0..39)
  y0[o]          = relu(sum_hf W0[o,hf,d] * z0[hf] + b0[o])   -> x1[b,o,d]
  z1[hf=h1*40+f] = x1[b,h1,d] * x[b,f,d]            (h1 in 0..63)
  y1[o]          = relu(sum_hf W1[o,hf,d] * z1[hf] + b1[o])   -> x2[b,o,d]
  out[b] = [sum_d x[b,:,d] | sum_d x1[b,:,d] | sum_d x2[b,:,d]]   (2048, 168)

Sharding: 4-way batch x 2-way embedding-channel split (8 cores); each core
computes partial d-sums for its 512-row shard over its 16 d-channels; the
host adds the two d-halves. Input dtypes preserved (fp32 in/out).

Design (vs the v1 selection-matmul baseline, ~9x fewer stall cycles):
  - Layer 1 is SYMMETRIZED on the host (W0s = W0 + W0^T off-diagonal): only
    the 820 unique (h<=f) products are computed, packed into 8 K-tiles with
    a fixed per-partition f-pattern (resident XF1); selection matmuls
    realize the per-tile h-patterns, so no AP-expressibility constraint
    applies. Layer-1 matmul passes drop 28 -> 16 per d-group.
  - z-tiles are built by one of three production paths, cycled per tile
    (PROD_CYCLE): F = selection-matmul into PSUM + fused DVE multiply
    directly from PSUM (no copy); S = selection-matmul + ScalarE copy +
    Pool multiply; D = partition-replicating DMA (stride-0 APs, spread over
    the SP/Act/Pool queues) + Pool multiply. Default is 80%% F / 20%% S: a
    DMA-free rep body is both fastest and robust to DMA-engine state
    (replication DMAs measured 1.9-20us/tile across device sessions).
  - The 248 K-tiles per rep are emitted as ONE flat software-pipelined
    stream: each tile's production is emitted LOOKAHEAD tiles ahead of its
    consuming matmuls, so the in-order engine queues never head-of-line
    block on cross-engine production chains (real-HW semaphore hops are
    ~1.5us; without lookahead every tile serialized at ~3.3us).
  - relu+bias fused as one (64,1024) ScalarE activation per (group, layer);
    d-sums accumulate in bf16 pair tiles, combined + PE-transposed in the
    epilogue.
"""

import os
from contextlib import ExitStack

import numpy as np
import ml_dtypes

import concourse.bass as bass
import concourse.bacc as bacc
import concourse.tile as tile
from concourse import mybir
from concourse.bass_utils import run_bass_kernel_spmd

BF16 = mybir.dt.bfloat16
FP32 = mybir.dt.float32
NPBF16 = ml_dtypes.bfloat16

B, F, D = 2048, 40, 32
O0, O1 = 64, 64
NCORES = 8
NB = 4                      # batch shards
ND = 2                      # d shards
BC = B // NB                # 512 batch rows per core
DC = D // ND                # 16 embedding channels per core
NT1 = 8                     # L1 K-tiles (symmetric triangular packing)
NT2 = 22                    # L2 K-tiles (3h x 40f packing)
DG = DC // 2                # 8 d-groups (2 d per group)
NCOL = 2 * BC               # 1024 free columns per group (d-major)
NMM = 512                   # max fp32-PSUM matmul free size
W = DC * BC                 # 8192 resident free width

# --- L1 triangular slot map (see _l1_slotmap) -----------------------------
def _l1_slotmap():
    # tile t covers h = t + HOFF[p]; segments j=0..4 of sizes 40/32/24/16/8
    # with f-runs [0..39], [8..39], [16..39], [24..39], [32..39]; [120:128)
    # pads carry W=0. Coverage: pair (h<=f) at t = h%8, j = h//8 (f >= h
    # guarantees f is within segment j's run). Selection matmuls realize
    # the h-pattern, so no AP-expressibility constraint applies.
    hoff = np.zeros(128, dtype=np.int64)
    f0 = np.zeros(128, dtype=np.int64)
    pad = np.zeros(128, dtype=bool)
    segs = [(0, 40, 0, 0), (40, 32, 8, 8), (72, 24, 16, 16),
            (96, 16, 24, 24), (112, 8, 32, 32), (120, 8, 32, 32)]
    for i, (base, n, ho, flo) in enumerate(segs):
        for k in range(n):
            p = base + k
            hoff[p] = ho
            f0[p] = flo + k
            pad[p] = (i == 5)
    return hoff, f0, pad


HOFF, F0, PAD0 = _l1_slotmap()

# DMA queue cycle (program order) and Pool multiply slots (tunable)
QUEUE_CYCLE = ["sp", "act", "sp", "act", "sp", "act", "pool"]
POOL_MUL_SLOTS = {2, 5, 8, 11, 13}   # of every 14 multiplies

# Production mode per tile, cycled in emission order (tunable).
#   F = sel-matmul -> PSUM, fused DVE multiply from PSUM
#   S = sel-matmul -> PSUM, ScalarE copy -> SBUF, Pool multiply
#   D = replication DMA -> SBUF, Pool multiply
PROD_CYCLE = "FFFSF"
Z_BUFS = 6                  # z ring depth (must exceed LOOKAHEAD)
XH_BUFS = 4
LOOKAHEAD = 6               # production lookahead; MUST be <= 9 (L1 block
                            # size), else L2 productions are emitted before
                            # relu1 and read stale x1t in program order


class _QueueSched:
    """Strict program-order round-robin over the 3 DMA queues."""

    def __init__(self, nc):
        self.eng = {"sp": nc.sync, "act": nc.scalar, "pool": nc.gpsimd}
        self.i = 0

    def pick(self):
        q = QUEUE_CYCLE[self.i % len(QUEUE_CYCLE)]
        self.i += 1
        return self.eng[q]


def _build_bass(reps=1):
    nc = bacc.Bacc()
    xt = nc.declare_dram_parameter("xt", [F, W], BF16, isOutput=False)
    w0t = nc.declare_dram_parameter("w0t", [128, NT1 * DC * O0], BF16, isOutput=False)
    w1t = nc.declare_dram_parameter("w1t", [128, NT2 * DC * O1], BF16, isOutput=False)
    sel0 = nc.declare_dram_parameter("sel0", [F, NT1 * 128], BF16, isOutput=False)
    sel1 = nc.declare_dram_parameter("sel1", [O0, NT2 * 128], BF16, isOutput=False)
    b0 = nc.declare_dram_parameter("b0", [O0, 1], FP32, isOutput=False)
    b1 = nc.declare_dram_parameter("b1", [O1, 1], FP32, isOutput=False)
    out = nc.declare_dram_parameter("out", [BC, O0 + O1], FP32, isOutput=True)

    with ExitStack() as ctx:
        tc = ctx.enter_context(tile.TileContext(nc))
        singles = ctx.enter_context(tc.tile_pool(name="singles", bufs=1))
        y_ps = ctx.enter_context(tc.tile_pool(name="y_ps", bufs=2, space="PSUM"))
        xh_ps = ctx.enter_context(tc.tile_pool(name="xh_ps", bufs=2, space="PSUM"))
        xh_sb = ctx.enter_context(tc.tile_pool(name="xh_sb", bufs=XH_BUFS))
        z_sb = ctx.enter_context(tc.tile_pool(name="z_sb", bufs=Z_BUFS))
        x2_sb = ctx.enter_context(tc.tile_pool(name="x2_sb", bufs=2))
        o_sb = ctx.enter_context(tc.tile_pool(name="o_sb", bufs=2))

        # ---- resident tensors ----
        xt_sb = singles.tile([F, W], BF16)
        sel0s = singles.tile([F, NT1, 128], BF16)
        sel1s = singles.tile([O0, NT2, 128], BF16)
        xf1 = singles.tile([128, W], BF16)
        xf2 = singles.tile([128, W], BF16)
        w0s = singles.tile([128, NT1, DC * O0], BF16)
        w1s = singles.tile([128, NT2, DC * O1], BF16)
        b0s = singles.tile([O0, 1], FP32)
        b1s = singles.tile([O1, 1], FP32)
        x1t = singles.tile([O0, W], BF16)
        acc1p = singles.tile([O0, NCOL], BF16)
        acc2p = singles.tile([O1, NCOL], BF16)
        acc1f = singles.tile([O0, BC], FP32)
        acc2f = singles.tile([O1, BC], FP32)

        from concourse.masks import make_identity
        ident = singles.tile([128, 128], FP32)
        make_identity(nc, ident)

        xt_ap = xt[:]

        def dram_rep(offset_elems, ap):
            return bass.AP(tensor=xt_ap.tensor, offset=xt_ap.offset + offset_elems,
                           ap=ap)

        def load_inputs():
            lq = _QueueSched(nc)
            lq.pick().dma_start(out=xt_sb, in_=xt[:])
            lq.pick().dma_start(out=sel0s, in_=sel0[:])
            lq.pick().dma_start(out=sel1s, in_=sel1[:])
            # XF2: partition p = rep*40 + f holds xT row f (pad: rows 0..7)
            lq.pick().dma_start(
                out=xf2[0:3 * F, :], in_=dram_rep(0, [[0, 3], [W, F], [1, W]])
            )
            lq.pick().dma_start(
                out=xf2[3 * F:128, :], in_=dram_rep(0, [[W, 128 - 3 * F], [1, W]])
            )
            # XF1: fixed f0 pattern, contiguous row-runs (+ one 2x8 run)
            runs = [(0, 0, 40), (40, 8, 32), (72, 16, 24), (96, 24, 16),
                    (112, 32, 8), (120, 32, 8)]
            for pstart, row0, n in runs:
                lq.pick().dma_start(
                    out=xf1[pstart:pstart + n, :],
                    in_=dram_rep(row0 * W, [[W, n], [1, W]]),
                )
            lq.pick().dma_start(out=w0s, in_=w0t[:])
            lq.pick().dma_start(out=w1s, in_=w1t[:])
            lq.pick().dma_start(out=b0s, in_=b0[:])
            lq.pick().dma_start(out=b1s, in_=b1[:])

        load_inputs()

        x1t_ap = x1t[:]

        for rep in range(reps):
            qs = _QueueSched(nc)
            prod_i = 0

            def produce(name, t, g, kp, dma_src, sels, rhs_sb, krows, xf):
                """Build z = XH * XF for one K-tile; returns the z tile.

                Production mode cycles through PROD_CYCLE: F = sel-matmul +
                fused DVE multiply from PSUM; S = sel-matmul + ScalarE copy
                + Pool multiply; D = replication DMA + Pool multiply.
                """
                nonlocal prod_i
                mode = PROD_CYCLE[prod_i % len(PROD_CYCLE)]
                if mode == "D" and dma_src is None:
                    mode = "F"
                prod_i += 1
                col0 = g * NCOL
                z = z_sb.tile([128, NCOL], BF16, tag="z", name=f"z{name}")
                xfs = xf[0:kp, col0:col0 + NCOL]
                if mode == "D":
                    xh = xh_sb.tile([128, NCOL], BF16, tag="xh",
                                    name=f"xh{name}")
                    qs.pick().dma_start(out=xh[0:kp, :], in_=dma_src)
                    nc.gpsimd.tensor_mul(z[0:kp, :], xh[0:kp, :], xfs)
                    return z
                xp = xh_ps.tile([128, NCOL], FP32, tag="xp", name=f"xp{name}")
                for i in range(2):
                    nc.tensor.matmul(
                        xp[:, i * NMM:(i + 1) * NMM],
                        lhsT=sels[:, t, :],
                        rhs=rhs_sb[0:krows, col0 + i * NMM:col0 + (i + 1) * NMM],
                        start=True,
                        stop=True,
                    )
                if mode == "F":
                    nc.vector.tensor_mul(z[0:kp, :], xp[0:kp, :], xfs)
                else:
                    xh = xh_sb.tile([128, NCOL], BF16, tag="xh",
                                    name=f"xh{name}")
                    nc.scalar.copy(out=xh[0:kp, :], in_=xp[0:kp, :])
                    nc.gpsimd.tensor_mul(z[0:kp, :], xh[0:kp, :], xfs)
                return z

            nc.vector.memset(acc1p, 0.0)
            nc.vector.memset(acc2p, 0.0)

            # ---- flat software-pipelined tile stream ----
            # Tile order interleaves groups (L1(0), L1(1), L2(0), L1(2),
            # L2(1), ..., L2(7)); z-productions are emitted LOOKAHEAD tiles
            # ahead of their consuming matmuls so the in-order engine queues
            # never stall on the cross-engine production chains.
            def l1_spec(g, t):
                return dict(name=f"1_{rep}_{g}_{t}", t=t, g=g, kp=128,
                            src=None, sels=sel0s, rhs=xt_sb, krows=F, xf=xf1,
                            ws=w0s, wk=128, blk=("L1", g), last=(t == NT1 - 1))

            def l2_spec(g, t):
                col0 = g * NCOL
                nh = 3 if t < NT2 - 1 else 1
                kp = nh * F
                src = bass.AP(tensor=x1t_ap.tensor,
                              offset=x1t_ap.offset + 3 * t * W + col0,
                              ap=[[W, nh], [0, F], [1, NCOL]])
                return dict(name=f"2_{rep}_{g}_{t}", t=t, g=g, kp=kp,
                            src=src, sels=sel1s, rhs=x1t, krows=O0, xf=xf2,
                            ws=w1s, wk=kp, blk=("L2", g), last=(t == NT2 - 1))

            seq = [l1_spec(0, t) for t in range(NT1)]
            seq += [l1_spec(1, t) for t in range(NT1)]
            for g in range(DG):
                seq += [l2_spec(g, t) for t in range(NT2)]
                if g + 2 < DG:
                    seq += [l1_spec(g + 2, t) for t in range(NT1)]

            yps = {}

            def finish_block(blk):
                kind, g = blk
                col0 = g * NCOL
                yp = yps.pop(blk)
                if kind == "L1":
                    nc.scalar.activation(
                        out=x1t[:, col0:col0 + NCOL], in_=yp,
                        func=mybir.ActivationFunctionType.Relu,
                        bias=b0s, scale=1.0,
                    )
                    nc.vector.tensor_add(acc1p, acc1p,
                                         x1t[:, col0:col0 + NCOL])
                else:
                    x2 = x2_sb.tile([O1, NCOL], BF16, tag="x2",
                                    name=f"x2_{rep}_{g}")
                    nc.scalar.activation(
                        out=x2, in_=yp,
                        func=mybir.ActivationFunctionType.Relu,
                        bias=b1s, scale=1.0,
                    )
                    nc.vector.tensor_add(acc2p, acc2p, x2)

            def consume(s, z):
                blk = s["blk"]
                if blk not in yps:
                    yps[blk] = y_ps.tile([O0, NCOL], FP32, tag="y",
                                         name=f"y{blk[0]}_{rep}_{blk[1]}")
                yp = yps[blk]
                kp = s["kp"]
                for i in range(2):
                    nc.tensor.matmul(
                        yp[:, i * NMM:(i + 1) * NMM],
                        lhsT=s["ws"][0:s["wk"], s["t"],
                                     (2 * s["g"] + i) * O0:
                                     (2 * s["g"] + i + 1) * O0],
                        rhs=z[0:kp, i * NMM:(i + 1) * NMM],
                        start=(s["t"] == 0),
                        stop=s["last"],
                    )
                if s["last"]:
                    finish_block(blk)

            zq = []
            for s in seq:
                z = produce(s["name"], s["t"], s["g"], s["kp"], s["src"],
                            s["sels"], s["rhs"], s["krows"], s["xf"])
                zq.append((s, z))
                if len(zq) > LOOKAHEAD:
                    sc, zc = zq.pop(0)
                    consume(sc, zc)
            for sc, zc in zq:
                consume(sc, zc)

            # ---- epilogue: combine bf16 d-half pairs, transpose, store ----
            nc.vector.tensor_add(acc1f, acc1p[:, 0:BC], acc1p[:, BC:NCOL])
            nc.vector.tensor_add(acc2f, acc2p[:, 0:BC], acc2p[:, BC:NCOL])
            for bh in range(BC // 128):
                outT = o_sb.tile([128, O0 + O1], FP32, tag="outT",
                                 name=f"outT_{rep}_{bh}")
                for acc, off in ((acc1f, 0), (acc2f, O0)):
                    pt = xh_ps.tile([128, NCOL], FP32, tag="xp",
                                    name=f"pt_{rep}_{bh}_{off}")
                    nc.tensor.transpose(
                        pt[:, 0:64], acc[:, bh * 128:(bh + 1) * 128],
                        ident[0:64, 0:64]
                    )
                    nc.vector.tensor_copy(out=outT[:, off:off + 64],
                                          in_=pt[:, 0:64])
                nc.sync.dma_start(
                    out=out[bh * 128:(bh + 1) * 128, :], in_=outT
                )

    nc.compile()
    return nc


_NC_CACHE = {}
LAST_RESULT = None


def _get_nc(reps=1):
    if reps not in _NC_CACHE:
        _NC_CACHE[reps] = _build_bass(reps)
    return _NC_CACHE[reps]


def _host_prep(x, W0, b0, W1, b1):
    """Build per-core input maps (host-side layout prep, cheap numpy)."""
    # ---- symmetrized L1 weights -> (128, NT1, DC, O0) tiles ----
    W0r = W0.reshape(O0, F, F, D)                       # (o, h, f, d)
    W0sym = W0r + W0r.transpose(0, 2, 1, 3)             # symmetric, h!=f summed
    di = np.arange(F)
    W0sym[:, di, di, :] = W0r[:, di, di, :]             # diagonal not doubled

    W1r = W1.reshape(O1, O0, F, D)                      # (o, h1, f, d)

    def prep_w0(dh):
        Wd = W0sym[:, :, :, dh * DC:(dh + 1) * DC]      # (o, h, f, DC)
        tiles = np.zeros((128, NT1, DC, O0), dtype=NPBF16)
        for t in range(NT1):
            h, f = t + HOFF, F0
            valid = (~PAD0) & (h <= f)
            blk = Wd[:, h[valid], f[valid], :]          # (o, nv, DC)
            tiles[valid, t] = blk.transpose(1, 2, 0).astype(NPBF16)
        return np.ascontiguousarray(tiles.reshape(128, NT1 * DC * O0))

    def prep_w1(dh):
        Wd = W1r[:, :, :, dh * DC:(dh + 1) * DC]        # (o, h1, f, DC)
        tiles = np.zeros((128, NT2, DC, O1), dtype=NPBF16)
        p = np.arange(120)
        for t in range(NT2):
            h = 3 * t + p // F
            f = p % F
            valid = h < O0
            blk = Wd[:, h[valid], f[valid], :]          # (o, nv, DC)
            tiles[p[valid], t] = blk.transpose(1, 2, 0).astype(NPBF16)
        return np.ascontiguousarray(tiles.reshape(128, NT2 * DC * O1))

    w_half = [(prep_w0(dh), prep_w1(dh)) for dh in range(ND)]
    b0h = b0.reshape(O0, 1).astype(np.float32)
    b1h = b1.reshape(O1, 1).astype(np.float32)

    # selection matrices: sel[k, t*128+p] = 1 iff h_t(p) == k
    sel0h = np.zeros((F, NT1, 128), dtype=NPBF16)
    for t in range(NT1):
        sel0h[t + HOFF, t, np.arange(128)] = 1.0
    sel1h = np.zeros((O0, NT2, 128), dtype=NPBF16)
    p120 = np.arange(120)
    for t in range(NT2):
        h = 3 * t + p120 // F
        m = h < O0
        sel1h[h[m], t, p120[m]] = 1.0
    sel0h = np.ascontiguousarray(sel0h.reshape(F, NT1 * 128))
    sel1h = np.ascontiguousarray(sel1h.reshape(O0, NT2 * 128))

    in_maps = []
    for c in range(NCORES):
        bs, dh = c % NB, c // NB
        xc = x[bs * BC:(bs + 1) * BC]                   # (512, 40, 32)
        xtc = np.ascontiguousarray(
            xc[:, :, dh * DC:(dh + 1) * DC].transpose(1, 2, 0).reshape(F, W)
        ).astype(NPBF16)
        in_maps.append({
            "xt": xtc,
            "w0t": w_half[dh][0],
            "w1t": w_half[dh][1],
            "sel0": sel0h,
            "sel1": sel1h,
            "b0": b0h,
            "b1": b1h,
        })
    return in_maps


def kernel(x, W0, b0, W1, b1):
    global LAST_RESULT
    x = np.asarray(x, dtype=np.float32)
    W0 = np.asarray(W0, dtype=np.float32)
    W1 = np.asarray(W1, dtype=np.float32)
    b0 = np.asarray(b0, dtype=np.float32)
    b1 = np.asarray(b1, dtype=np.float32)

    nc = _get_nc()
    in_maps = _host_prep(x, W0, b0, W1, b1)
    res = run_bass_kernel_spmd(nc, in_maps, core_ids=list(range(NCORES)))
    LAST_RESULT = res

    out = np.empty((B, F + O0 + O1), dtype=np.float32)
    out[:, :F] = x.sum(axis=-1)
    for bs in range(NB):
        half0 = np.asarray(res.results[bs]["out"])
        half1 = np.asarray(res.results[NB + bs]["out"])
        out[bs * BC:(bs + 1) * BC, F:] = half0 + half1
    return out
